# revision 70
# baseline (speedup 1.0000x reference)
"""Trainium2 Bass kernel for BiBo attention (GQA + per-head RMSNorm + RoPE +
SSMax scaling + causal attention + o_proj).

Sharding: tensor-parallel over the 4 KV-head groups x data-parallel over the
2 batch elements = 8 cores. Each core computes its 4 q-heads / 1 kv-head of
attention for one batch element plus its row-slice of o_proj; the host sums
the 4 partial o_proj outputs per batch element (row-parallel unshard).

Layout strategy (per core):
  - hidden^T [H, S] streamed from DRAM; projections produce q^T/k^T with the
    head dim on partitions so QK^T needs no transposes.
  - scores are computed transposed (scoresT[k, q]) so the PV matmul consumes
    exp(scoresT) directly; the softmax denominator is a ones-vector matmul
    (partition-dim sum on the PE) over quad-summed exp tiles; no
    max-subtraction is needed because RMS-normed q/k bound
    |scores| <= sqrt(HD)*ssmax*log(S) ~ 10.
  - causal structure: blocks fully below the diagonal are computed without
    any mask work; fully-masked blocks are skipped; the 4 diagonal blocks of
    each q-tile share one constant 128x128 triangular 0/1 bf16 mask applied
    to the exp tile on the vector engine, with QK/exp/PV narrowed to the
    live columns. Non-causal masks fall back to a generic additive path.
  - rstd = exp(-0.5*ln(var*sc + b)) on the scalar engine so the whole kernel
    uses a single activation table set (ln+exp); sqrt would thrash the
    table RAMs against exp.
"""

import math

import numpy as np

B, S, H = 2, 2048, 2048
NH, NKV, HD = 16, 4, 128
EPS = 1e-6
NCORES = 8
TP = 4            # kv-head groups
QH = NH // NKV    # q heads per core
SC = 512          # q-tile / s-chunk width
NSC = S // SC     # 4
KT = 128          # k tile
NKT = S // KT     # 16
HC = 128          # h contraction chunk
NHC = H // HC     # 16
SKIP_THRESH = -1e8

_compiled_cache = {}
LAST_EXEC_NS = None
LAST_RESULTS = None


def _enable_ldw_opt():
    import os
    if not os.environ.get("BASS_LDW_OPT"):
        return
    from concourse import bass_utils as bu
    if getattr(bu.run_command, "_ldw_patched", False):
        return
    orig = bu.run_command

    def patched(argv, **kw):
        argv = ["--enable-ldw-opt=true" if a == "--enable-ldw-opt=false" else a
                for a in argv]
        return orig(argv, **kw)

    patched._ldw_patched = True
    bu.run_command = patched


def _pin_act_table(arch, AF):
    """Restrict the activation-table chooser to the one set containing both
    ln and exp, so Ln/Exp/Square/Copy alternation never reloads tables.
    Mutates the functools-cached dict in place (emptied entries keep their
    index so act_func_set_id stays aligned with act_info.json)."""
    from concourse.hw_specs import get_activation_tables
    tabs = get_activation_tables(arch)
    keep = "natural_log_exp_and_others"
    needed = {AF.Exp, AF.Ln, AF.Square, AF.Copy}
    if keep in tabs and needed <= tabs[keep]:
        for name in list(tabs):
            if name != keep:
                tabs[name] = set()


def _build_program(plan, mask_counts):
    import concourse.mybir as mybir
    import concourse.tile as tile
    from concourse import bacc

    F32 = mybir.dt.float32
    MM = mybir.dt.bfloat16
    AF = mybir.ActivationFunctionType
    OP = mybir.AluOpType

    n_mask = sum(mask_counts)

    _enable_ldw_opt()
    nc = bacc.Bacc("TRN2", target_bir_lowering=False, debug=False,
                   num_devices=NCORES)
    _pin_act_table(nc.m.arch, AF)
    hT = nc.dram_tensor("hT", [NSC, 4, HC, 4 * SC], MM,
                        kind="ExternalInput").ap()
    wqT = nc.dram_tensor("wqT", [QH, HC, NHC * HD], MM,
                         kind="ExternalInput").ap()
    wkT = nc.dram_tensor("wkT", [HC, NHC * HD], MM,
                         kind="ExternalInput").ap()
    wvT = nc.dram_tensor("wvT", [HC, NHC * HD], MM,
                         kind="ExternalInput").ap()
    woT = nc.dram_tensor("woT", [HD, QH * H], MM, kind="ExternalInput").ap()
    cosT = nc.dram_tensor("cosT", [HD, S], F32,
                          kind="ExternalInput").ap()
    sinT = nc.dram_tensor("sinT", [HD, S], MM,
                          kind="ExternalInput").ap()
    cbf = nc.dram_tensor("cbf", [128, KT + 2], MM, kind="ExternalInput").ap()
    cqs = nc.dram_tensor("cqs", [1, 2 * (QH + 1)], F32,
                         kind="ExternalInput").ap()
    if n_mask:
        mblk = nc.dram_tensor("mblk", [n_mask, KT, SC], F32,
                              kind="ExternalInput").ap()
    out = nc.dram_tensor("out", [S, H], MM, kind="ExternalOutput").ap()

    with tile.TileContext(nc) as tc:
        _emit(nc, tc, locals(), plan, mask_counts, MM, F32, AF, OP)
    nc.compile()
    return nc


def _emit(nc, tc, T, plan, mask_counts, MM, F32, AF, OP):
    from contextlib import ExitStack

    hT, wqT, woT = T["hT"], T["wqT"], T["woT"]
    wkT, wvT = T["wkT"], T["wvT"]
    cosT, sinT = T["cosT"], T["sinT"]
    cbf, cqs, out = T["cbf"], T["cqs"], T["out"]
    mblk = T.get("mblk")

    ctx = ExitStack()
    with ctx:
        const = ctx.enter_context(tc.tile_pool(name="const", bufs=1))
        wpool = ctx.enter_context(tc.tile_pool(name="w", bufs=1))
        persist = ctx.enter_context(tc.tile_pool(name="persist", bufs=1))
        hpool = ctx.enter_context(tc.tile_pool(name="h", bufs=8))
        mpool = ctx.enter_context(tc.tile_pool(name="m", bufs=4))
        spool = ctx.enter_context(tc.tile_pool(name="s", bufs=2))
        epool = ctx.enter_context(tc.tile_pool(name="e", bufs=3))
        atpool = ctx.enter_context(tc.tile_pool(name="at", bufs=8))
        opool_sb = ctx.enter_context(tc.tile_pool(name="osb", bufs=6))
        ps_mm = ctx.enter_context(tc.tile_pool(name="psmm", bufs=3, space="PSUM"))
        ps_pv = ctx.enter_context(tc.tile_pool(name="pspv", bufs=2, space="PSUM"))
        ps_sm = ctx.enter_context(tc.tile_pool(name="pssm", bufs=1, space="PSUM"))
        ps_o = ctx.enter_context(tc.tile_pool(name="pso", bufs=2, space="PSUM"))

        # ---- persistent tiles (loads emitted by the driver below) -------
        # wq is head-major so head 0's projection can start before the
        # later heads' weights arrive
        wq_h = [wpool.tile([128, NHC * HD], MM, tag=f"wqh{h}", name=f"wqh{h}")
                for h in range(QH)]
        wk_t = wpool.tile([128, NHC * HD], MM, tag="wk")
        wv_t = wpool.tile([128, NHC * HD], MM, tag="wv")
        wo_t = wpool.tile([128, QH * H], MM, tag="wo")
        cos_t = wpool.tile([128, NSC * SC], F32, tag="cos")
        sin_t = wpool.tile([128, NSC * SC], MM, tag="sin")
        cs_loaded = [False] * NSC

        def cos_sl(sc):
            return cos_t[:, sc * SC:(sc + 1) * SC]

        def sin_sl(sc):
            return sin_t[:, sc * SC:(sc + 1) * SC]

        def load_cs(sc):
            if not cs_loaded[sc]:
                nc.sync.dma_start(cos_t[:, sc * SC:(sc + 1) * SC],
                                  cosT[:, sc * SC:(sc + 1) * SC])
                nc.sync.dma_start(sin_t[:, sc * SC:(sc + 1) * SC],
                                  sinT[:, sc * SC:(sc + 1) * SC])
                cs_loaded[sc] = True

        cbf_t = const.tile([128, KT + 2], MM, tag="cbf")
        tri_t = cbf_t[:, 0:KT]
        iwq_t = cbf_t[:, KT:KT + 1]
        iwk_t = cbf_t[:, KT + 1:KT + 2]
        cqs_t = const.tile([1, 2 * (QH + 1)], F32, tag="cqs")
        qsc_t = cqs_t[:, 0:QH + 1]
        qsb_t = cqs_t[:, QH + 1:2 * (QH + 1)]
        ones_t = const.tile([128, 1], MM, tag="ones")
        khat = persist.tile([128, S], MM, tag="khat")
        v_sb = persist.tile([128, S], MM, tag="v")
        qhat = [persist.tile([128, S], MM, name=f"qhat{i}", tag=f"qhat{i}")
                for i in range(QH)]

        def load_rest():
            nc.sync.dma_start(wv_t[:], wvT[:])
            nc.sync.dma_start(wq_h[0][:], wqT[0])
            load_cs(0)
            for h in range(1, QH):
                nc.sync.dma_start(wq_h[h][:], wqT[h])
            nc.sync.dma_start(cbf_t[:], cbf[:])
            nc.sync.dma_start(cqs_t[:], cqs[:])
            nc.vector.memset(ones_t[:], 1.0)

        def load_wo():
            nc.sync.dma_start(wo_t[:], woT[:])

        # norm+rope staged: s1 (right after the proj matmuls) does the
        # rotate-copy + cos-product (the last PSUM reads, so the proj bank
        # frees early) and the square on the scalar engine; the var matmul
        # (s2) trails by one projection group; s3 finishes rstd + rope.
        def norm_s1(pp, sc):
            sh = spool.tile([128, SC], MM, tag="sh", name="sh", bufs=4)
            nc.vector.tensor_copy(sh[0:64, :], pp[64:128, :])
            nc.vector.tensor_copy(sh[64:128, :], pp[0:64, :])
            sq = spool.tile([128, SC], MM, tag="sq", name="sq", bufs=4)
            nc.scalar.activation(sq[:], pp[:], AF.Square)
            uu = spool.tile([128, SC], MM, tag="uu", name="uu", bufs=4)
            nc.vector.tensor_mul(uu[:], pp[:], cos_sl(sc))
            return sh, sq, uu

        def norm_s2(sq, iw_t):
            var = ps_mm.tile([1, SC], F32, tag="mm", name="var")
            nc.tensor.matmul(var[:], iw_t[:], sq[:], start=True, stop=True)
            return var

        def norm_s3(sh, uu, var, sc, hd, hat_dst):
            # rstd = (var*qsc + qsb)^-0.5 via ln+exp (single act table set)
            hi = 0 if hd is None else hd + 1
            lv = spool.tile([1, SC], F32, tag="lv", name="lv")
            nc.scalar.activation(lv[:], var[:], AF.Ln,
                                 bias=qsb_t[:, hi:hi + 1],
                                 scale=qsc_t[:, hi:hi + 1])
            rs = spool.tile([1, SC], MM, tag="rs", name="rs")
            nc.scalar.activation(rs[:], lv[:], AF.Exp, scale=-0.5)
            # tt = rot(x) * sin' (sign pre-folded into sin'), s = uu + tt
            tt = spool.tile([128, SC], MM, tag="tt", name="tt")
            nc.vector.tensor_mul(tt[:], sh[:], sin_sl(sc))
            bb = spool.tile([128, SC], MM, tag="bb", name="bb")
            nc.gpsimd.partition_broadcast(bb[:], rs[:], 128)
            nc.vector.tensor_add(tt[:], tt[:], uu[:])
            nc.vector.tensor_mul(hat_dst, tt[:], bb[:])

        # ---- projections, per s-chunk -----------------------------------
        def hts_load(sc):
            tiles = []
            for g in range(4):
                t = hpool.tile([128, 4 * SC], MM, tag="ht", name="ht")
                nc.sync.dma_start(t[:], hT[sc, g])
                tiles.append(t)
            return [tiles[c // 4][:, (c % 4) * SC:(c % 4 + 1) * SC]
                    for c in range(NHC)]

        def proj_chunk(sc, hts, carry=None, pending_out=None):
            # GENERATOR: yields after each unit (k, v, q0..q3) so the driver
            # can interleave projection units with attention units. The last
            # two finishers are appended to pending_out (not emitted) so the
            # next phase can interleave their M=1 var matmuls into its PE
            # stream instead of head-of-line blocking on Square.
            specs = [(iwk_t, None, khat)] + [
                (iwq_t, hd, qhat[hd]) for hd in range(QH)]
            state = []  # (sh, sq, uu, spec)

            def do_mm(idx):
                pp = ps_mm.tile([128, SC], F32, tag="mm", name="pp")
                for c in range(NHC):
                    if idx == 0:
                        w_sl = wk_t[:, c * HD:(c + 1) * HD]
                    else:
                        w_sl = wq_h[idx - 1][:, c * HD:(c + 1) * HD]
                    nc.tensor.matmul(pp[:], w_sl, hts[c][:],
                                     start=(c == 0), stop=(c == NHC - 1))
                sh, sq, uu = norm_s1(pp, sc)
                state.append((sh, sq, uu, specs[idx]))

            def finish_one():
                sh, sq, uu, (iw_t, hd, dst) = state.pop(0)
                var = norm_s2(sq, iw_t)
                norm_s3(sh, uu, var, sc, hd, dst[:, sc * SC:(sc + 1) * SC])

            def v_proj():
                for ss in range(4):
                    vp = ps_o.tile([128, SC], F32, tag="o", name="vp")
                    for c in range(NHC):
                        nc.tensor.matmul(vp[:, 0:HD],
                                         hts[c][:, ss * 128:(ss + 1) * 128],
                                         wv_t[:, c * HD:(c + 1) * HD],
                                         start=(c == 0), stop=(c == NHC - 1))
                    col = (sc * 4 + ss) * 128
                    nc.vector.tensor_copy(v_sb[:, col:col + 128], vp[:, 0:HD])

            do_mm(0)
            if carry:
                for fin in carry:
                    fin()
            yield
            v_proj()
            yield
            for idx in range(1, 5):
                do_mm(idx)
                if idx <= 3:
                    finish_one()
                yield
            if pending_out is not None:
                pending_out.extend([finish_one, finish_one])

        # ---- attention + o_proj, per q-tile ------------------------------
        mask_starts = [sum(mask_counts[:i]) for i in range(NSC)]

        def attn_qtile(qi, carry=None):
            # GENERATOR: yields after each head and after each o_proj half
            ats = []
            mask_idx = mask_starts[qi]
            kts = [kt for kt in range(NKT) if plan[qi][kt] != "skip"]
            mtiles = {}
            for kt in kts:
                if plan[qi][kt] == "mask":
                    mt = mpool.tile([128, SC], F32, tag="mask", name="mk")
                    nc.sync.dma_start(mt[:], mblk[mask_idx])
                    mtiles[kt] = mt
                    mask_idx += 1
            for hd in range(QH):
                qsl = qhat[hd][:, qi * SC:(qi + 1) * SC]
                pv = ps_pv.tile([128, SC], F32, tag="pv")
                es = ps_sm.tile([1, SC], F32, tag="es")
                sts = {}
                pend = []
                esn = [0, 0]  # groups emitted, total groups
                GRP = 8
                ngroups = (len(kts) + GRP - 1) // GRP
                esn[1] = ngroups

                def tail(j):
                    kt = kts[j]
                    st, c0 = sts.pop(j)
                    ex = epool.tile([128, SC], MM, tag="ex", name="ex",
                                    bufs=9)
                    nc.scalar.activation(ex[:, c0:SC], st[:, c0:SC], AF.Exp)
                    if c0:
                        nc.vector.memset(ex[:, 0:c0], 0.0)
                        nc.vector.tensor_mul(ex[:, c0:c0 + KT],
                                             ex[:, c0:c0 + KT], tri_t[:])
                    elif plan[qi][kt] == "diag0":
                        nc.vector.tensor_mul(ex[:, 0:KT], ex[:, 0:KT],
                                             tri_t[:])
                    last = j == len(kts) - 1
                    nc.tensor.matmul(pv[:, c0:SC],
                                     v_sb[:, kt * 128:(kt + 1) * 128],
                                     ex[:, c0:SC], start=(j == 0), stop=last)
                    pend.append(ex)
                    if len(pend) == GRP or last:
                        # pairwise add-tree -> one ones-matmul per group
                        cur = list(pend)
                        lvl = 0
                        while len(cur) > 1:
                            nxt = []
                            for a, b in zip(cur[0::2], cur[1::2]):
                                r = epool.tile([128, SC], MM,
                                               tag=f"exs{lvl}",
                                               name=f"exs{lvl}", bufs=4)
                                nc.vector.tensor_add(r[:], a[:], b[:])
                                nxt.append(r)
                            if len(cur) % 2:
                                nxt.append(cur[-1])
                            cur = nxt
                            lvl += 1
                        nc.tensor.matmul(es[:], ones_t[:], cur[0][:],
                                         start=(esn[0] == 0),
                                         stop=(esn[0] == esn[1] - 1))
                        esn[0] += 1
                        pend.clear()

                # pipeline QK^T one k-tile ahead of exp/PV
                for j, kt in enumerate(kts):
                    kind = plan[qi][kt]
                    c0 = int(kind[4]) * KT if kind.startswith("diag") else 0
                    st = ps_mm.tile([128, SC], F32, tag="mm")
                    nc.tensor.matmul(st[:, c0:SC],
                                     khat[:, kt * 128:(kt + 1) * 128],
                                     qsl[:, c0:SC], start=True, stop=True)
                    if kind == "mask":
                        nc.vector.tensor_add(st[:], st[:], mtiles[kt][:])
                    sts[j] = (st, c0)
                    if j >= 1:
                        tail(j - 1)
                tail(len(kts) - 1)
                rs = spool.tile([1, SC], F32, tag="ars")
                nc.vector.reciprocal_approx_fast(rs[:], es[:])
                bb = spool.tile([128, SC], F32, tag="abb")
                nc.gpsimd.partition_broadcast(bb[:], rs[:], 128)
                at = atpool.tile([128, SC], MM, tag="at")
                nc.vector.tensor_mul(at[:], pv[:], bb[:])
                ats.append(at)
                if hd == 0 and carry:
                    for fin in carry:
                        fin()
                    carry = None
                yield
            # o_proj for this q-tile. The PE is in-order, so the first
            # tile's head-3 matmul would stall ~3us on the last head's
            # es->recip->broadcast->at chain; borrow the (idle) pv PSUM
            # banks and pre-emit 4 tiles' head-0..2 partial sums as cover.
            def op_mm(t, hd, ss, ho, start, stop):
                nc.tensor.matmul(
                    t[:], ats[hd][:, ss * 128:(ss + 1) * 128],
                    wo_t[:, hd * H + ho * SC:hd * H + (ho + 1) * SC],
                    start=start, stop=stop)

            def op_tile(i):
                pool = ps_o if i % 2 == 0 else ps_pv
                return pool.tile([128, SC], F32,
                                 tag="o" if i % 2 == 0 else "pv", name="op")

            tiles_plan = [(ss, ho) for ss in range(4) for ho in range(4)]
            PRE = 4
            pre_tiles = []
            for i, (ss, ho) in enumerate(tiles_plan[:PRE]):
                t = op_tile(i)
                for hd in range(QH - 1):
                    op_mm(t, hd, ss, ho, hd == 0, False)
                pre_tiles.append(t)
            obs = {}
            tail_dmas = []
            for i, (ss, ho) in enumerate(tiles_plan):
                if i < PRE:
                    t = pre_tiles[i]
                    op_mm(t, QH - 1, ss, ho, False, True)
                else:
                    t = op_tile(i)
                    for hd in range(QH):
                        op_mm(t, hd, ss, ho, hd == 0, hd == QH - 1)
                if ss not in obs:
                    obs[ss] = opool_sb.tile([128, H], MM, tag="osb",
                                            name="ob")
                ob = obs[ss]
                if ho % 2 == 0:
                    nc.scalar.copy(ob[:, ho * SC:(ho + 1) * SC], t[:])
                else:
                    nc.vector.tensor_copy(ob[:, ho * SC:(ho + 1) * SC],
                                          t[:])
                if ho == 3:
                    dst = out[qi * SC + ss * 128:qi * SC + (ss + 1) * 128, :]
                    if qi == NSC - 1:
                        # defer HWDGE issues past all copies: a waiting
                        # dma_start head-of-line blocks the scalar queue
                        tail_dmas.append((dst, ob))
                    else:
                        nc.gpsimd.dma_start(dst, ob[:])
                if i == 7:
                    yield
            for dst, ob in tail_dmas:
                nc.scalar.dma_start(dst, ob[:])

        # ---- driver: software-pipelined phase order ----------------------
        # DMA order = first-use order: wk, hT chunk0 (4 groups so the k-proj
        # streams per-group), wv, wq, cos/sin/consts, then the rest.
        nc.sync.dma_start(wk_t[:], wkT[:])
        hts0 = hts_load(0)
        load_rest()
        # HAM warmup: ~3.5us of dummy matmuls on memset data while the
        # first DMAs land, so the real projections start at 2.4GHz instead
        # of the cold 1.2GHz gate
        warm = spool.tile([128, SC], MM, tag="sh", name="warm", bufs=4)
        nc.vector.memset(warm[:], 0.0)
        wp = ps_o.tile([128, SC], F32, tag="o", name="warmp")
        for i in range(9):
            nc.tensor.matmul(wp[:], warm[:, 0:128], warm[:],
                             start=(i == 0), stop=(i == 8))
        def interleave(*gens):
            alive = list(gens)
            while alive:
                for g in list(alive):
                    try:
                        next(g)
                    except StopIteration:
                        alive.remove(g)

        def drain(g):
            interleave(g)

        p0, p1, p2, p3 = [], [], [], []
        drain(proj_chunk(0, hts0, pending_out=p0))
        hts1 = hts_load(1)
        load_cs(1)
        drain(proj_chunk(1, hts1, carry=p0, pending_out=p1))
        load_wo()
        hts2 = hts_load(2)
        load_cs(2)
        # zip attention with the next chunk's projections: each phase's
        # latency chains are covered by the other's matmul stream
        interleave(attn_qtile(0, carry=p1),
                   proj_chunk(2, hts2, pending_out=p2))
        hts3 = hts_load(3)
        load_cs(3)
        interleave(attn_qtile(1, carry=p2),
                   proj_chunk(3, hts3, pending_out=p3))
        interleave(attn_qtile(2, carry=p3), attn_qtile(3))


def _causal_diag_j(blk, qi, kt):
    """Return j in 0..3 if the block matches the canonical causal step at
    diagonal offset (kt == 4*qi + j), else None. blk: [B, SC, KT]."""
    j = kt - 4 * qi
    if not (0 <= j <= 3):
        return None
    q_idx = qi * SC + np.arange(SC)[:, None]
    k_idx = kt * KT + np.arange(KT)[None, :]
    want = np.where(k_idx > q_idx, np.float32(-1e9), np.float32(0.0))
    return j if bool((blk == want[None]).all()) else None


def _mask_plan(mask):
    """Classify [qi][kt] blocks of the (q,k) mask, unified across batch."""
    plan = []
    for qi in range(NSC):
        row = []
        for kt in range(NKT):
            blk = mask[:, 0, qi * SC:(qi + 1) * SC, kt * KT:(kt + 1) * KT]
            if (blk <= SKIP_THRESH).all():
                row.append("skip")
            elif (blk == 0.0).all():
                row.append("zero")
            else:
                j = _causal_diag_j(blk, qi, kt)
                row.append(f"diag{j}" if j is not None else "mask")
        # guard: a q-tile with no included block would divide by zero
        if all(s == "skip" for s in row):
            row[0] = "mask"
        plan.append(row)
    return plan


def kernel(hidden_states, cos, sin, attention_mask, wq, wk, wv, wo,
           q_norm_w, k_norm_w, ssmax_scale):
    global LAST_EXEC_NS
    import os
    import ml_dtypes
    from concourse.bass_utils import run_bass_kernel_spmd

    f32 = np.float32
    hidden_states = np.asarray(hidden_states, f32)
    cos = np.asarray(cos, f32)
    sin = np.asarray(sin, f32)
    attention_mask = np.asarray(attention_mask, f32)
    wq = np.asarray(wq, f32)
    wk = np.asarray(wk, f32)
    wv = np.asarray(wv, f32)
    wo = np.asarray(wo, f32)
    q_norm_w = np.asarray(q_norm_w, f32)
    k_norm_w = np.asarray(k_norm_w, f32)
    ssmax = np.asarray(ssmax_scale, f32).reshape(NH)

    plan = _mask_plan(attention_mask)
    mask_counts = [sum(1 for s in row if s == "mask") for row in plan]
    key = (tuple(tuple(r) for r in plan),)
    if key not in _compiled_cache:
        _compiled_cache[key] = _build_program(plan, mask_counts)
    nc = _compiled_cache[key]

    bf16 = ml_dtypes.bfloat16
    qw = np.tile(q_norm_w, QH)
    iwq_np = (1.0 / (HD * q_norm_w ** 2)).astype(bf16)[:, None]
    iwk_np = (1.0 / (HD * k_norm_w ** 2)).astype(bf16)[:, None]
    tri_np = (np.arange(KT)[:, None] <= np.arange(KT)[None, :]).astype(bf16)
    cbf_np = np.concatenate([tri_np, iwq_np, iwk_np], axis=1)  # [128, KT+2]
    # cos kept f32; sin gets rotate_half's sign fold: sin'[d<64] = -sin[d]
    sinp = sin.T.copy()                       # [HD, S]
    sinp[:64] = -sinp[:64]
    cos_np = np.ascontiguousarray(cos.T)                       # [HD, S]
    sin_np = np.ascontiguousarray(sinp).astype(bf16)

    in_maps = []
    for core in range(NCORES):
        b, g = divmod(core, TP)
        hTm = np.ascontiguousarray(
            hidden_states[b].T.reshape(4, 4, HC, NSC, SC)
            .transpose(3, 0, 2, 1, 4).reshape(NSC, 4, HC, 4 * SC)
        ).astype(bf16)
        wq_s = wq[g * QH * HD:(g + 1) * QH * HD] * qw[:, None]
        wk_s = wk[g * HD:(g + 1) * HD] * k_norm_w[:, None]
        wv_s = wv[g * HD:(g + 1) * HD]
        wo_s = wo[:, g * QH * HD:(g + 1) * QH * HD]
        qcv = np.array([ssmax[g * QH + i] * math.log(S) / math.sqrt(HD)
                        for i in range(QH)], f32)
        # entry 0 is the k-norm (qc=1); entries 1..QH are the q heads
        qcall = np.concatenate([[1.0], qcv]).astype(f32)
        cqs_np = np.concatenate(
            [1.0 / qcall ** 2, EPS / qcall ** 2])[None, :].astype(f32)
        # wq head-major: [hd][128(h-in-chunk), c*HD + d]
        wqTm = np.ascontiguousarray(
            wq_s.T.reshape(NHC, HC, QH, HD)
            .transpose(2, 1, 0, 3).reshape(QH, HC, NHC * HD)).astype(bf16)
        wkTm = np.ascontiguousarray(
            wk_s.T.reshape(NHC, HC, HD)
            .transpose(1, 0, 2).reshape(HC, NHC * HD)).astype(bf16)
        wvTm = np.ascontiguousarray(
            wv_s.T.reshape(NHC, HC, HD)
            .transpose(1, 0, 2).reshape(HC, NHC * HD)).astype(bf16)
        # wo SBUF layout: [128(d-in-head), hd*H + hcol]
        woTm = np.ascontiguousarray(
            wo_s.T.reshape(QH, HD, H)
            .transpose(1, 0, 2).reshape(HD, QH * H)).astype(bf16)
        m = {
            "hT": hTm,
            "wqT": wqTm,
            "wkT": wkTm,
            "wvT": wvTm,
            "woT": woTm,
            "cosT": cos_np, "sinT": sin_np,
            "cbf": cbf_np, "cqs": cqs_np,
        }
        n_mask = sum(mask_counts)
        if n_mask:
            blocks = np.zeros((n_mask, KT, SC), f32)
            i = 0
            for qi in range(NSC):
                for kt in range(NKT):
                    if plan[qi][kt] != "mask":
                        continue
                    blocks[i] = attention_mask[
                        b, 0, qi * SC:(qi + 1) * SC,
                        kt * KT:(kt + 1) * KT].T
                    i += 1
            m["mblk"] = blocks
        in_maps.append(m)

    trace = bool(int(os.environ.get("BASS_KERNEL_TRACE", "0")))
    res = run_bass_kernel_spmd(nc, in_maps, list(range(NCORES)), trace=trace)
    LAST_EXEC_NS = res.exec_time_ns
    globals()["LAST_RESULTS"] = res

    final = np.zeros((B, S, H), f32)
    for core in range(NCORES):
        b = core // TP
        final[b] += np.asarray(res.results[core]["out"], f32)
    return final


# revision 72
# speedup vs baseline: 1.1863x; 1.1863x over previous
"""Trainium2 Bass kernel for BiBo attention (GQA + per-head RMSNorm + RoPE +
SSMax scaling + causal attention + o_proj).

Sharding: tensor-parallel over the 4 KV-head groups x data-parallel over the
2 batch elements = 8 cores. Each core computes its 4 q-heads / 1 kv-head of
attention for one batch element plus its row-slice of o_proj; the host sums
the 4 partial o_proj outputs per batch element (row-parallel unshard).

Layout strategy (per core):
  - hidden^T [H, S] streamed from DRAM; projections produce q^T/k^T with the
    head dim on partitions so QK^T needs no transposes.
  - scores are computed transposed (scoresT[k, q]) so the PV matmul consumes
    exp(scoresT) directly; the softmax denominator is a ones-vector matmul
    (partition-dim sum on the PE) over quad-summed exp tiles; no
    max-subtraction is needed because RMS-normed q/k bound
    |scores| <= sqrt(HD)*ssmax*log(S) ~ 10.
  - causal structure: blocks fully below the diagonal are computed without
    any mask work; fully-masked blocks are skipped; the 4 diagonal blocks of
    each q-tile share one constant 128x128 triangular 0/1 bf16 mask applied
    to the exp tile on the vector engine, with QK/exp/PV narrowed to the
    live columns. Non-causal masks fall back to a generic additive path.
  - rstd = exp(-0.5*ln(var*sc + b)) on the scalar engine so the whole kernel
    uses a single activation table set (ln+exp); sqrt would thrash the
    table RAMs against exp.
"""

import math

import numpy as np

B, S, H = 2, 2048, 2048
NH, NKV, HD = 16, 4, 128
EPS = 1e-6
NCORES = 8
TP = 4            # kv-head groups
QH = NH // NKV    # q heads per core
SC = 512          # q-tile / s-chunk width
NSC = S // SC     # 4
KT = 128          # k tile
NKT = S // KT     # 16
HC = 128          # h contraction chunk
NHC = H // HC     # 16
SKIP_THRESH = -1e8

_compiled_cache = {}
LAST_EXEC_NS = None
LAST_RESULTS = None


def _enable_ldw_opt():
    import os
    if not os.environ.get("BASS_LDW_OPT"):
        return
    from concourse import bass_utils as bu
    if getattr(bu.run_command, "_ldw_patched", False):
        return
    orig = bu.run_command

    def patched(argv, **kw):
        argv = ["--enable-ldw-opt=true" if a == "--enable-ldw-opt=false" else a
                for a in argv]
        return orig(argv, **kw)

    patched._ldw_patched = True
    bu.run_command = patched


def _pin_act_table(arch, AF):
    """Restrict the activation-table chooser to the one set containing both
    ln and exp, so Ln/Exp/Square/Copy alternation never reloads tables.
    Mutates the functools-cached dict in place (emptied entries keep their
    index so act_func_set_id stays aligned with act_info.json)."""
    from concourse.hw_specs import get_activation_tables
    tabs = get_activation_tables(arch)
    keep = "natural_log_exp_and_others"
    needed = {AF.Exp, AF.Ln, AF.Square, AF.Copy}
    if keep in tabs and needed <= tabs[keep]:
        for name in list(tabs):
            if name != keep:
                tabs[name] = set()


def _build_program(plan, mask_counts):
    import concourse.mybir as mybir
    import concourse.tile as tile
    from concourse import bacc

    F32 = mybir.dt.float32
    MM = mybir.dt.bfloat16
    AF = mybir.ActivationFunctionType
    OP = mybir.AluOpType

    n_mask = sum(mask_counts)

    _enable_ldw_opt()
    nc = bacc.Bacc("TRN2", target_bir_lowering=False, debug=False,
                   num_devices=NCORES)
    _pin_act_table(nc.m.arch, AF)
    hT = nc.dram_tensor("hT", [NSC, 4, HC, 4 * SC], MM,
                        kind="ExternalInput").ap()
    wqT = nc.dram_tensor("wqT", [QH, HC, NHC * HD], MM,
                         kind="ExternalInput").ap()
    wkT = nc.dram_tensor("wkT", [HC, NHC * HD], MM,
                         kind="ExternalInput").ap()
    wvT = nc.dram_tensor("wvT", [HC, NHC * HD], MM,
                         kind="ExternalInput").ap()
    woT = nc.dram_tensor("woT", [HD, QH * H], MM, kind="ExternalInput").ap()
    cosT = nc.dram_tensor("cosT", [HD, S], F32,
                          kind="ExternalInput").ap()
    sinT = nc.dram_tensor("sinT", [HD, S], MM,
                          kind="ExternalInput").ap()
    cbf = nc.dram_tensor("cbf", [128, KT + 2], MM, kind="ExternalInput").ap()
    cqs = nc.dram_tensor("cqs", [1, 2 * (QH + 1)], F32,
                         kind="ExternalInput").ap()
    if n_mask:
        mblk = nc.dram_tensor("mblk", [n_mask, KT, SC], F32,
                              kind="ExternalInput").ap()
    out = nc.dram_tensor("out", [S, H], MM, kind="ExternalOutput").ap()

    with tile.TileContext(nc) as tc:
        _emit(nc, tc, locals(), plan, mask_counts, MM, F32, AF, OP)
    nc.compile()
    return nc


def _emit(nc, tc, T, plan, mask_counts, MM, F32, AF, OP):
    from contextlib import ExitStack

    hT, wqT, woT = T["hT"], T["wqT"], T["woT"]
    wkT, wvT = T["wkT"], T["wvT"]
    cosT, sinT = T["cosT"], T["sinT"]
    cbf, cqs, out = T["cbf"], T["cqs"], T["out"]
    mblk = T.get("mblk")

    ctx = ExitStack()
    with ctx:
        const = ctx.enter_context(tc.tile_pool(name="const", bufs=1))
        wpool = ctx.enter_context(tc.tile_pool(name="w", bufs=1))
        persist = ctx.enter_context(tc.tile_pool(name="persist", bufs=1))
        hpool = ctx.enter_context(tc.tile_pool(name="h", bufs=8))
        mpool = ctx.enter_context(tc.tile_pool(name="m", bufs=4))
        spool = ctx.enter_context(tc.tile_pool(name="s", bufs=2))
        epool = ctx.enter_context(tc.tile_pool(name="e", bufs=3))
        atpool = ctx.enter_context(tc.tile_pool(name="at", bufs=8))
        opool_sb = ctx.enter_context(tc.tile_pool(name="osb", bufs=6))
        ps_mm = ctx.enter_context(tc.tile_pool(name="psmm", bufs=3, space="PSUM"))
        ps_pv = ctx.enter_context(tc.tile_pool(name="pspv", bufs=2, space="PSUM"))
        ps_sm = ctx.enter_context(tc.tile_pool(name="pssm", bufs=1, space="PSUM"))
        ps_o = ctx.enter_context(tc.tile_pool(name="pso", bufs=2, space="PSUM"))

        # ---- persistent tiles (loads emitted by the driver below) -------
        # wq is head-major so head 0's projection can start before the
        # later heads' weights arrive
        wq_h = [wpool.tile([128, NHC * HD], MM, tag=f"wqh{h}", name=f"wqh{h}")
                for h in range(QH)]
        wk_t = wpool.tile([128, NHC * HD], MM, tag="wk")
        wv_t = wpool.tile([128, NHC * HD], MM, tag="wv")
        wo_t = wpool.tile([128, QH * H], MM, tag="wo")
        cos_t = wpool.tile([128, NSC * SC], F32, tag="cos")
        sin_t = wpool.tile([128, NSC * SC], MM, tag="sin")
        cs_loaded = [False] * NSC

        def cos_sl(sc):
            return cos_t[:, sc * SC:(sc + 1) * SC]

        def sin_sl(sc):
            return sin_t[:, sc * SC:(sc + 1) * SC]

        def load_cs(sc):
            if not cs_loaded[sc]:
                nc.sync.dma_start(cos_t[:, sc * SC:(sc + 1) * SC],
                                  cosT[:, sc * SC:(sc + 1) * SC])
                nc.sync.dma_start(sin_t[:, sc * SC:(sc + 1) * SC],
                                  sinT[:, sc * SC:(sc + 1) * SC])
                cs_loaded[sc] = True

        cbf_t = const.tile([128, KT + 2], MM, tag="cbf")
        tri_t = cbf_t[:, 0:KT]
        iwq_t = cbf_t[:, KT:KT + 1]
        iwk_t = cbf_t[:, KT + 1:KT + 2]
        cqs_t = const.tile([1, 2 * (QH + 1)], F32, tag="cqs")
        qsc_t = cqs_t[:, 0:QH + 1]
        qsb_t = cqs_t[:, QH + 1:2 * (QH + 1)]
        ones_t = const.tile([128, 1], MM, tag="ones")
        khat = persist.tile([128, S], MM, tag="khat")
        v_sb = persist.tile([128, S], MM, tag="v")
        qhat = [persist.tile([128, S], MM, name=f"qhat{i}", tag=f"qhat{i}")
                for i in range(QH)]

        def load_rest():
            nc.sync.dma_start(wv_t[:], wvT[:])
            nc.sync.dma_start(wq_h[0][:], wqT[0])
            load_cs(0)
            for h in range(1, QH):
                nc.sync.dma_start(wq_h[h][:], wqT[h])
            nc.sync.dma_start(cbf_t[:], cbf[:])
            nc.sync.dma_start(cqs_t[:], cqs[:])
            nc.vector.memset(ones_t[:], 1.0)

        def load_wo():
            nc.sync.dma_start(wo_t[:], woT[:])

        # norm+rope staged: s1 (right after the proj matmuls) does the
        # rotate-copy + cos-product (the last PSUM reads, so the proj bank
        # frees early) and the square on the scalar engine; the var matmul
        # (s2) trails by one projection group; s3 finishes rstd + rope.
        def norm_s1(pp, sc):
            sh = spool.tile([128, SC], MM, tag="sh", name="sh", bufs=4)
            nc.vector.tensor_copy(sh[0:64, :], pp[64:128, :])
            nc.vector.tensor_copy(sh[64:128, :], pp[0:64, :])
            sq = spool.tile([128, SC], MM, tag="sq", name="sq", bufs=4)
            nc.scalar.activation(sq[:], pp[:], AF.Square)
            uu = spool.tile([128, SC], MM, tag="uu", name="uu", bufs=4)
            nc.vector.tensor_mul(uu[:], pp[:], cos_sl(sc))
            return sh, sq, uu

        def norm_s2(sq, iw_t):
            var = ps_mm.tile([1, SC], F32, tag="mm", name="var")
            nc.tensor.matmul(var[:], iw_t[:], sq[:], start=True, stop=True)
            return var

        def norm_s3(sh, uu, var, sc, hd, hat_dst):
            # rstd = (var*qsc + qsb)^-0.5 via ln+exp (single act table set)
            hi = 0 if hd is None else hd + 1
            lv = spool.tile([1, SC], F32, tag="lv", name="lv")
            nc.scalar.activation(lv[:], var[:], AF.Ln,
                                 bias=qsb_t[:, hi:hi + 1],
                                 scale=qsc_t[:, hi:hi + 1])
            rs = spool.tile([1, SC], MM, tag="rs", name="rs")
            nc.scalar.activation(rs[:], lv[:], AF.Exp, scale=-0.5)
            # tt = rot(x) * sin' (sign pre-folded into sin'), s = uu + tt
            tt = spool.tile([128, SC], MM, tag="tt", name="tt")
            nc.vector.tensor_mul(tt[:], sh[:], sin_sl(sc))
            bb = spool.tile([128, SC], MM, tag="bb", name="bb")
            nc.gpsimd.partition_broadcast(bb[:], rs[:], 128)
            nc.vector.tensor_add(tt[:], tt[:], uu[:])
            nc.vector.tensor_mul(hat_dst, tt[:], bb[:])

        # ---- projections, per s-chunk -----------------------------------
        def hts_load(sc):
            tiles = []
            for g in range(4):
                t = hpool.tile([128, 4 * SC], MM, tag="ht", name="ht")
                nc.sync.dma_start(t[:], hT[sc, g])
                tiles.append(t)
            return [tiles[c // 4][:, (c % 4) * SC:(c % 4 + 1) * SC]
                    for c in range(NHC)]

        def proj_chunk(sc, hts, carry=None, pending_out=None):
            # GENERATOR: yields after each unit (k, v, q0..q3) so the driver
            # can interleave projection units with attention units. The last
            # two finishers are appended to pending_out (not emitted) so the
            # next phase can interleave their M=1 var matmuls into its PE
            # stream instead of head-of-line blocking on Square.
            specs = [(iwk_t, None, khat)] + [
                (iwq_t, hd, qhat[hd]) for hd in range(QH)]
            state = []  # (sh, sq, uu, spec)

            def do_mm(idx):
                pp = ps_mm.tile([128, SC], F32, tag="mm", name="pp")
                for c in range(NHC):
                    if idx == 0:
                        w_sl = wk_t[:, c * HD:(c + 1) * HD]
                    else:
                        w_sl = wq_h[idx - 1][:, c * HD:(c + 1) * HD]
                    nc.tensor.matmul(pp[:], w_sl, hts[c][:],
                                     start=(c == 0), stop=(c == NHC - 1))
                sh, sq, uu = norm_s1(pp, sc)
                state.append((sh, sq, uu, specs[idx]))

            def finish_one():
                sh, sq, uu, (iw_t, hd, dst) = state.pop(0)
                var = norm_s2(sq, iw_t)
                norm_s3(sh, uu, var, sc, hd, dst[:, sc * SC:(sc + 1) * SC])

            def v_proj():
                for ss in range(4):
                    vp = ps_o.tile([128, SC], F32, tag="o", name="vp")
                    for c in range(NHC):
                        nc.tensor.matmul(vp[:, 0:HD],
                                         hts[c][:, ss * 128:(ss + 1) * 128],
                                         wv_t[:, c * HD:(c + 1) * HD],
                                         start=(c == 0), stop=(c == NHC - 1))
                    col = (sc * 4 + ss) * 128
                    nc.vector.tensor_copy(v_sb[:, col:col + 128], vp[:, 0:HD])

            do_mm(0)
            if carry:
                for fin in carry:
                    fin()
            yield
            v_proj()
            yield
            for idx in range(1, 5):
                do_mm(idx)
                if idx <= 3:
                    finish_one()
                yield
            if pending_out is not None:
                pending_out.extend([finish_one, finish_one])

        # ---- attention + o_proj, per q-tile ------------------------------
        mask_starts = [sum(mask_counts[:i]) for i in range(NSC)]

        def attn_qtile(qi, carry=None):
            # GENERATOR: yields after each head and after each o_proj half
            ats = []
            mask_idx = mask_starts[qi]
            kts = [kt for kt in range(NKT) if plan[qi][kt] != "skip"]
            mtiles = {}
            for kt in kts:
                if plan[qi][kt] == "mask":
                    mt = mpool.tile([128, SC], F32, tag="mask", name="mk")
                    nc.sync.dma_start(mt[:], mblk[mask_idx])
                    mtiles[kt] = mt
                    mask_idx += 1
            for hd in range(QH):
                qsl = qhat[hd][:, qi * SC:(qi + 1) * SC]
                pv = ps_pv.tile([128, SC], F32, tag="pv")
                es = ps_sm.tile([1, SC], F32, tag="es")
                sts = {}
                pend = []
                esn = [0, 0]  # groups emitted, total groups
                GRP = 4
                ngroups = (len(kts) + GRP - 1) // GRP
                esn[1] = ngroups

                def tail(j):
                    kt = kts[j]
                    st, c0 = sts.pop(j)
                    ex = epool.tile([128, SC], MM, tag="ex", name="ex",
                                    bufs=5)
                    nc.scalar.activation(ex[:, c0:SC], st[:, c0:SC], AF.Exp)
                    if c0:
                        nc.vector.memset(ex[:, 0:c0], 0.0)
                        nc.vector.tensor_mul(ex[:, c0:c0 + KT],
                                             ex[:, c0:c0 + KT], tri_t[:])
                    elif plan[qi][kt] == "diag0":
                        nc.vector.tensor_mul(ex[:, 0:KT], ex[:, 0:KT],
                                             tri_t[:])
                    last = j == len(kts) - 1
                    nc.tensor.matmul(pv[:, c0:SC],
                                     v_sb[:, kt * 128:(kt + 1) * 128],
                                     ex[:, c0:SC], start=(j == 0), stop=last)
                    pend.append(ex)
                    if len(pend) == GRP or last:
                        # pairwise add-tree -> one ones-matmul per group
                        cur = list(pend)
                        lvl = 0
                        while len(cur) > 1:
                            nxt = []
                            for a, b in zip(cur[0::2], cur[1::2]):
                                r = epool.tile([128, SC], MM,
                                               tag=f"exs{lvl}",
                                               name=f"exs{lvl}", bufs=4)
                                nc.vector.tensor_add(r[:], a[:], b[:])
                                nxt.append(r)
                            if len(cur) % 2:
                                nxt.append(cur[-1])
                            cur = nxt
                            lvl += 1
                        nc.tensor.matmul(es[:], ones_t[:], cur[0][:],
                                         start=(esn[0] == 0),
                                         stop=(esn[0] == esn[1] - 1))
                        esn[0] += 1
                        pend.clear()

                # pipeline QK^T one k-tile ahead of exp/PV
                for j, kt in enumerate(kts):
                    kind = plan[qi][kt]
                    c0 = int(kind[4]) * KT if kind.startswith("diag") else 0
                    st = ps_mm.tile([128, SC], F32, tag="mm")
                    nc.tensor.matmul(st[:, c0:SC],
                                     khat[:, kt * 128:(kt + 1) * 128],
                                     qsl[:, c0:SC], start=True, stop=True)
                    if kind == "mask":
                        nc.vector.tensor_add(st[:], st[:], mtiles[kt][:])
                    sts[j] = (st, c0)
                    if j >= 1:
                        tail(j - 1)
                tail(len(kts) - 1)
                rs = spool.tile([1, SC], F32, tag="ars")
                nc.vector.reciprocal_approx_fast(rs[:], es[:])
                bb = spool.tile([128, SC], F32, tag="abb")
                nc.gpsimd.partition_broadcast(bb[:], rs[:], 128)
                at = atpool.tile([128, SC], MM, tag="at")
                nc.vector.tensor_mul(at[:], pv[:], bb[:])
                ats.append(at)
                if hd == 0 and carry:
                    for fin in carry:
                        fin()
                    carry = None
                yield
            # o_proj for this q-tile. The PE is in-order, so the first
            # tile's head-3 matmul would stall ~3us on the last head's
            # es->recip->broadcast->at chain; borrow the (idle) pv PSUM
            # banks and pre-emit 4 tiles' head-0..2 partial sums as cover.
            def op_mm(t, hd, ss, ho, start, stop):
                nc.tensor.matmul(
                    t[:], ats[hd][:, ss * 128:(ss + 1) * 128],
                    wo_t[:, hd * H + ho * SC:hd * H + (ho + 1) * SC],
                    start=start, stop=stop)

            def op_tile(i):
                pool = ps_o if i % 2 == 0 else ps_pv
                return pool.tile([128, SC], F32,
                                 tag="o" if i % 2 == 0 else "pv", name="op")

            tiles_plan = [(ss, ho) for ss in range(4) for ho in range(4)]
            PRE = 4
            pre_tiles = []
            for i, (ss, ho) in enumerate(tiles_plan[:PRE]):
                t = op_tile(i)
                for hd in range(QH - 1):
                    op_mm(t, hd, ss, ho, hd == 0, False)
                pre_tiles.append(t)
            obs = {}
            tail_dmas = []
            for i, (ss, ho) in enumerate(tiles_plan):
                if i < PRE:
                    t = pre_tiles[i]
                    op_mm(t, QH - 1, ss, ho, False, True)
                else:
                    t = op_tile(i)
                    for hd in range(QH):
                        op_mm(t, hd, ss, ho, hd == 0, hd == QH - 1)
                if ss not in obs:
                    obs[ss] = opool_sb.tile([128, H], MM, tag="osb",
                                            name="ob")
                ob = obs[ss]
                if ho % 2 == 0:
                    nc.scalar.copy(ob[:, ho * SC:(ho + 1) * SC], t[:])
                else:
                    nc.vector.tensor_copy(ob[:, ho * SC:(ho + 1) * SC],
                                          t[:])
                if ho == 3:
                    dst = out[qi * SC + ss * 128:qi * SC + (ss + 1) * 128, :]
                    if qi == NSC - 1:
                        # defer HWDGE issues past all copies: a waiting
                        # dma_start head-of-line blocks the scalar queue
                        tail_dmas.append((dst, ob))
                    else:
                        nc.gpsimd.dma_start(dst, ob[:])
                if i == 7:
                    yield
            for dst, ob in tail_dmas:
                nc.scalar.dma_start(dst, ob[:])

        # ---- driver: software-pipelined phase order ----------------------
        # DMA order = first-use order: wk, hT chunk0 (4 groups so the k-proj
        # streams per-group), wv, wq, cos/sin/consts, then the rest.
        nc.sync.dma_start(wk_t[:], wkT[:])
        hts0 = hts_load(0)
        load_rest()
        # HAM warmup: ~3.5us of dummy matmuls on memset data while the
        # first DMAs land, so the real projections start at 2.4GHz instead
        # of the cold 1.2GHz gate
        warm = spool.tile([128, SC], MM, tag="sh", name="warm", bufs=4)
        nc.vector.memset(warm[:], 0.0)
        wp = ps_o.tile([128, SC], F32, tag="o", name="warmp")
        for i in range(9):
            nc.tensor.matmul(wp[:], warm[:, 0:128], warm[:],
                             start=(i == 0), stop=(i == 8))
        def interleave(*gens):
            alive = list(gens)
            while alive:
                for g in list(alive):
                    try:
                        next(g)
                    except StopIteration:
                        alive.remove(g)

        def drain(g):
            interleave(g)

        p0, p1, p2, p3 = [], [], [], []
        drain(proj_chunk(0, hts0, pending_out=p0))
        hts1 = hts_load(1)
        load_cs(1)
        drain(proj_chunk(1, hts1, carry=p0, pending_out=p1))
        load_wo()
        hts2 = hts_load(2)
        load_cs(2)
        # zip attention with the next chunk's projections: each phase's
        # latency chains are covered by the other's matmul stream
        interleave(attn_qtile(0, carry=p1),
                   proj_chunk(2, hts2, pending_out=p2))
        hts3 = hts_load(3)
        load_cs(3)
        interleave(attn_qtile(1, carry=p2),
                   proj_chunk(3, hts3, pending_out=p3))
        interleave(attn_qtile(2, carry=p3), attn_qtile(3))


def _causal_diag_j(blk, qi, kt):
    """Return j in 0..3 if the block matches the canonical causal step at
    diagonal offset (kt == 4*qi + j), else None. blk: [B, SC, KT]."""
    j = kt - 4 * qi
    if not (0 <= j <= 3):
        return None
    q_idx = qi * SC + np.arange(SC)[:, None]
    k_idx = kt * KT + np.arange(KT)[None, :]
    want = np.where(k_idx > q_idx, np.float32(-1e9), np.float32(0.0))
    return j if bool((blk == want[None]).all()) else None


def _mask_plan(mask):
    """Classify [qi][kt] blocks of the (q,k) mask, unified across batch."""
    plan = []
    for qi in range(NSC):
        row = []
        for kt in range(NKT):
            blk = mask[:, 0, qi * SC:(qi + 1) * SC, kt * KT:(kt + 1) * KT]
            if (blk <= SKIP_THRESH).all():
                row.append("skip")
            elif (blk == 0.0).all():
                row.append("zero")
            else:
                j = _causal_diag_j(blk, qi, kt)
                row.append(f"diag{j}" if j is not None else "mask")
        # guard: a q-tile with no included block would divide by zero
        if all(s == "skip" for s in row):
            row[0] = "mask"
        plan.append(row)
    return plan


def kernel(hidden_states, cos, sin, attention_mask, wq, wk, wv, wo,
           q_norm_w, k_norm_w, ssmax_scale):
    global LAST_EXEC_NS
    import os
    import ml_dtypes
    from concourse.bass_utils import run_bass_kernel_spmd

    f32 = np.float32
    hidden_states = np.asarray(hidden_states, f32)
    cos = np.asarray(cos, f32)
    sin = np.asarray(sin, f32)
    attention_mask = np.asarray(attention_mask, f32)
    wq = np.asarray(wq, f32)
    wk = np.asarray(wk, f32)
    wv = np.asarray(wv, f32)
    wo = np.asarray(wo, f32)
    q_norm_w = np.asarray(q_norm_w, f32)
    k_norm_w = np.asarray(k_norm_w, f32)
    ssmax = np.asarray(ssmax_scale, f32).reshape(NH)

    plan = _mask_plan(attention_mask)
    mask_counts = [sum(1 for s in row if s == "mask") for row in plan]
    key = (tuple(tuple(r) for r in plan),)
    if key not in _compiled_cache:
        _compiled_cache[key] = _build_program(plan, mask_counts)
    nc = _compiled_cache[key]

    bf16 = ml_dtypes.bfloat16
    qw = np.tile(q_norm_w, QH)
    iwq_np = (1.0 / (HD * q_norm_w ** 2)).astype(bf16)[:, None]
    iwk_np = (1.0 / (HD * k_norm_w ** 2)).astype(bf16)[:, None]
    tri_np = (np.arange(KT)[:, None] <= np.arange(KT)[None, :]).astype(bf16)
    cbf_np = np.concatenate([tri_np, iwq_np, iwk_np], axis=1)  # [128, KT+2]
    # cos kept f32; sin gets rotate_half's sign fold: sin'[d<64] = -sin[d]
    sinp = sin.T.copy()                       # [HD, S]
    sinp[:64] = -sinp[:64]
    cos_np = np.ascontiguousarray(cos.T)                       # [HD, S]
    sin_np = np.ascontiguousarray(sinp).astype(bf16)

    in_maps = []
    for core in range(NCORES):
        b, g = divmod(core, TP)
        hTm = np.ascontiguousarray(
            hidden_states[b].T.reshape(4, 4, HC, NSC, SC)
            .transpose(3, 0, 2, 1, 4).reshape(NSC, 4, HC, 4 * SC)
        ).astype(bf16)
        wq_s = wq[g * QH * HD:(g + 1) * QH * HD] * qw[:, None]
        wk_s = wk[g * HD:(g + 1) * HD] * k_norm_w[:, None]
        wv_s = wv[g * HD:(g + 1) * HD]
        wo_s = wo[:, g * QH * HD:(g + 1) * QH * HD]
        qcv = np.array([ssmax[g * QH + i] * math.log(S) / math.sqrt(HD)
                        for i in range(QH)], f32)
        # entry 0 is the k-norm (qc=1); entries 1..QH are the q heads
        qcall = np.concatenate([[1.0], qcv]).astype(f32)
        cqs_np = np.concatenate(
            [1.0 / qcall ** 2, EPS / qcall ** 2])[None, :].astype(f32)
        # wq head-major: [hd][128(h-in-chunk), c*HD + d]
        wqTm = np.ascontiguousarray(
            wq_s.T.reshape(NHC, HC, QH, HD)
            .transpose(2, 1, 0, 3).reshape(QH, HC, NHC * HD)).astype(bf16)
        wkTm = np.ascontiguousarray(
            wk_s.T.reshape(NHC, HC, HD)
            .transpose(1, 0, 2).reshape(HC, NHC * HD)).astype(bf16)
        wvTm = np.ascontiguousarray(
            wv_s.T.reshape(NHC, HC, HD)
            .transpose(1, 0, 2).reshape(HC, NHC * HD)).astype(bf16)
        # wo SBUF layout: [128(d-in-head), hd*H + hcol]
        woTm = np.ascontiguousarray(
            wo_s.T.reshape(QH, HD, H)
            .transpose(1, 0, 2).reshape(HD, QH * H)).astype(bf16)
        m = {
            "hT": hTm,
            "wqT": wqTm,
            "wkT": wkTm,
            "wvT": wvTm,
            "woT": woTm,
            "cosT": cos_np, "sinT": sin_np,
            "cbf": cbf_np, "cqs": cqs_np,
        }
        n_mask = sum(mask_counts)
        if n_mask:
            blocks = np.zeros((n_mask, KT, SC), f32)
            i = 0
            for qi in range(NSC):
                for kt in range(NKT):
                    if plan[qi][kt] != "mask":
                        continue
                    blocks[i] = attention_mask[
                        b, 0, qi * SC:(qi + 1) * SC,
                        kt * KT:(kt + 1) * KT].T
                    i += 1
            m["mblk"] = blocks
        in_maps.append(m)

    trace = bool(int(os.environ.get("BASS_KERNEL_TRACE", "0")))
    res = run_bass_kernel_spmd(nc, in_maps, list(range(NCORES)), trace=trace)
    LAST_EXEC_NS = res.exec_time_ns
    globals()["LAST_RESULTS"] = res

    final = np.zeros((B, S, H), f32)
    for core in range(NCORES):
        b = core // TP
        final[b] += np.asarray(res.results[core]["out"], f32)
    return final
